# revision 12
# baseline (speedup 1.0000x reference)
"""Trainium2 Bass kernel for nn_BandProcessor — v3 (engine-rebalanced).

Structure per 128-token tile (64 tiles/core, batch data-parallel over 8 cores):
  L1: LN1 -> temporal 16-tap causal band (PE band matmul + N=15 spill) ->
      proj (+bias in PSUM) -> x residual added during PSUM evac -> x1
  L2: LN2 -> 3-tap neighbor band (N=1 edge columns) -> proj (+bias) ->
      x1 residual on evac -> x2
  FFN: LN3 -> transpose -> W1+gelu -> W2 (+bias +x2 residual matmul) -> out

Engine assignment (vs v2, which bottlenecked DVE at ~95%):
  - DVE: grouped bn_stats (one [128,2,256] call per LN per pair),
    tensor_tensor PSUM evacs that fold in the x/x1 residual adds
  - ACT: a1/a2/xn3T/out evacs (Copy), gelu (bias rides the activation)
  - GPSIMD: all three LN normalizes, plus batched mean/var combine and
    bit-trick rsqrt (RB=8 pairs per batch) - replaces all bn_aggr calls
  - PE: band/proj/FFN matmuls + ones-row bias matmuls + out residual;
    the f32 x-residual identity matmul is gone (folded into DVE evac)
  - PSUM pools sized to exactly 8 banks; every PSUM tile is bank-exclusive
"""

import numpy as np
import ml_dtypes

import concourse.bacc as bacc
import concourse.mybir as mybir
from concourse.tile import TileContext
from concourse import bass_utils

B, T, D = 8, 8192, 256
H = 16
DECAY = 0.9
EPS = 1e-5
NT = T // 128           # 64 token tiles per core
NP = NT // 2            # 32 pairs
SBP = 8                 # pairs per superblock (FFN batching) -> 2048 tokens
NSB = NP // SBP         # 4 superblocks
RB = 4                  # pairs per stats batch (8 tiles)

F32 = mybir.dt.float32
F32R = mybir.dt.float32r
BF16 = mybir.dt.bfloat16
I32 = mybir.dt.int32

AF = mybir.ActivationFunctionType
ALU = mybir.AluOpType

# engine knobs (quick rebalance without restructuring)
E_NORM1 = "gpsimd"
E_NORM23 = "vector"
E_BATCH = "gpsimd"


# ---------------------------------------------------------------- host prep

def _host_consts(inp, gelu_ok=True):
    """Fold LN gains + value/out projections into single matrices (f64)."""
    g1, b1_ = inp["n1_g"].astype(np.float64), inp["n1_b"].astype(np.float64)
    g2, b2_ = inp["n2_g"].astype(np.float64), inp["n2_b"].astype(np.float64)
    g3, b3_ = inp["n3_g"].astype(np.float64), inp["n3_b"].astype(np.float64)
    t_Wv, t_bv = inp["t_Wv"].astype(np.float64), inp["t_bv"].astype(np.float64)
    t_Wo, t_bo = inp["t_Wo"].astype(np.float64), inp["t_bo"].astype(np.float64)
    a_Wv, a_bv = inp["a_Wv"].astype(np.float64), inp["a_bv"].astype(np.float64)
    a_Wo, a_bo = inp["a_Wo"].astype(np.float64), inp["a_bo"].astype(np.float64)
    f_W1, f_b1 = inp["f_W1"].astype(np.float64), inp["f_b1"].astype(np.float64)
    f_W2, f_b2 = inp["f_W2"].astype(np.float64), inp["f_b2"].astype(np.float64)

    WtWo = t_Wv @ t_Wo
    WaWo = a_Wv @ a_Wo
    Wt_eff = g1[:, None] * WtWo                                # [D, D]
    bt_eff = b1_ @ WtWo + t_bv @ t_Wo + t_bo
    Wa_eff = g2[:, None] * WaWo
    ba_eff = b2_ @ WaWo + a_bv @ a_Wo + a_bo
    W1_eff = g3[:, None] * f_W1                                # [D, 2D]
    b1_eff = b3_ @ f_W1 + f_b1                                 # [2D]
    W2 = f_W2
    b2v = f_b2

    # temporal weights: reference tw[j] applies to h_pad[j:j+T];
    # lag d = H-1-j  ->  w_lag[d] = tw[H-1-d]
    tw = DECAY ** np.arange(H, dtype=np.float64)
    tw = tw / tw.sum()
    w_lag = tw[::-1].copy()

    band1c = np.zeros((128, 128), np.float64)
    for ti in range(128):
        for to in range(ti, min(128, ti + H)):
            band1c[ti, to] = w_lag[to - ti]
    # spill into next tile: cols 0..14 get lags 1..15 from prev tile's
    # partitions 113..127
    band1p15 = np.zeros((128, 15), np.float64)
    for p in range(113, 128):
        for to in range(0, p - 112):
            band1p15[p, to] = w_lag[to + 128 - p]
    band2c = np.zeros((128, 128), np.float64)
    for ti in range(128):
        for to in range(max(0, ti - 1), min(128, ti + 2)):
            band2c[ti, to] = 1.0 / 3.0
    ep_col = np.zeros((128, 1), np.float64); ep_col[127, 0] = 1.0 / 3.0
    ep0_col = np.zeros((128, 1), np.float64); ep0_col[0, 0] = 1.0 / 3.0
    en_col = np.zeros((128, 1), np.float64); en_col[0, 0] = 1.0 / 3.0
    en63_col = np.zeros((128, 1), np.float64); en63_col[127, 0] = 1.0 / 3.0

    bf = lambda a: np.ascontiguousarray(a).astype(ml_dtypes.bfloat16)
    f32 = lambda a: np.ascontiguousarray(a).astype(np.float32)

    # first-tile correction for the temporal zero-pad of the LN bias term
    c_t = np.cumsum(w_lag)[:H - 1]
    corr = f32((c_t - 1.0)[:, None] * (b1_ @ WtWo)[None, :])

    consts = {
        "wt": bf(np.stack([Wt_eff[0:128], Wt_eff[128:256]])),       # [2,128,256]
        "wa": bf(np.stack([Wa_eff[0:128], Wa_eff[128:256]])),
        "w1": bf(np.stack([W1_eff[0:128], W1_eff[128:256]])),       # [2,128,512]
        "w2": bf(np.stack([W2[k * 128:(k + 1) * 128] for k in range(4)])),
        "band1c": bf(band1c), "band1p15": bf(band1p15),
        "band2c": bf(band2c),
        "ecols": bf(np.concatenate([ep_col, ep0_col, en_col, en63_col], axis=1)),
        "ones_r": bf(np.ones((1, 128))),
        # biases duplicated for pair-wide N=512 matmuls
        "browp": bf(np.stack([np.tile(bt_eff, 2), np.tile(ba_eff, 2),
                              np.tile(b2v, 2)])),                   # [3,512]
        "b1col": f32(b1_eff.reshape(4, 128).T),                     # [128,4]
        "identb": bf(np.eye(128)),
    }
    need_corr = bool(np.abs(corr).max() > 0)
    return consts, corr, need_corr


# ---------------------------------------------------------------- bass build

def build_nc(repeat=1, need_corr=False, gelu=True):
    nc = bacc.Bacc("TRN2", target_bir_lowering=False, debug=False, num_devices=8)
    GELU = AF.Gelu if gelu else AF.Identity

    x_d = nc.dram_tensor("x", (T, D), F32R, kind="ExternalInput")
    out_d = nc.dram_tensor("out", (T, D), F32, kind="ExternalOutput")
    wt_d = nc.dram_tensor("wt", (2, 128, 256), BF16, kind="ExternalInput")
    wa_d = nc.dram_tensor("wa", (2, 128, 256), BF16, kind="ExternalInput")
    w1_d = nc.dram_tensor("w1", (2, 128, 512), BF16, kind="ExternalInput")
    w2_d = nc.dram_tensor("w2", (4, 128, 256), BF16, kind="ExternalInput")
    b1c_d = nc.dram_tensor("band1c", (128, 128), BF16, kind="ExternalInput")
    b1p_d = nc.dram_tensor("band1p15", (128, 15), BF16, kind="ExternalInput")
    b2c_d = nc.dram_tensor("band2c", (128, 128), BF16, kind="ExternalInput")
    ec_d = nc.dram_tensor("ecols", (128, 4), BF16, kind="ExternalInput")
    ones_d = nc.dram_tensor("ones_r", (1, 128), BF16, kind="ExternalInput")
    browp_d = nc.dram_tensor("browp", (3, 512), BF16, kind="ExternalInput")
    b1col_d = nc.dram_tensor("b1col", (128, 4), F32, kind="ExternalInput")
    idb_d = nc.dram_tensor("identb", (128, 128), BF16, kind="ExternalInput")
    corr_d = nc.dram_tensor("corr", (15, 256), F32, kind="ExternalInput") if need_corr else None

    def eng(name):
        return {"vector": nc.vector, "gpsimd": nc.gpsimd}[name]

    with TileContext(nc) as tc:
        import contextlib
        ctx = contextlib.ExitStack()
        with ctx:
            consts = ctx.enter_context(tc.tile_pool(name="consts", bufs=1))
            xpool = ctx.enter_context(tc.tile_pool(name="xpool", bufs=12))
            xn1p = ctx.enter_context(tc.tile_pool(name="xn1p", bufs=8))
            a1p = ctx.enter_context(tc.tile_pool(name="a1p", bufs=3))
            x1p = ctx.enter_context(tc.tile_pool(name="x1p", bufs=12))
            xn2p = ctx.enter_context(tc.tile_pool(name="xn2p", bufs=8))
            a2p = ctx.enter_context(tc.tile_pool(name="a2p", bufs=3))
            x2p = ctx.enter_context(tc.tile_pool(name="x2p", bufs=14))
            xn3p = ctx.enter_context(tc.tile_pool(name="xn3p", bufs=8))
            statp = ctx.enter_context(tc.tile_pool(name="statp", bufs=2))
            bigp = ctx.enter_context(tc.tile_pool(name="bigp", bufs=2))
            gelup = ctx.enter_context(tc.tile_pool(name="gelup", bufs=2))
            outp = ctx.enter_context(tc.tile_pool(name="outp", bufs=3))
            smalls = ctx.enter_context(tc.tile_pool(name="smalls", bufs=2))
            # PSUM: 8 banks total, every tile 1 full bank; per-tag bufs:
            # aggT 2 + att1 1 + att2 1 + att3 2 + gps 2 = 8
            pps = ctx.enter_context(tc.tile_pool(name="pps", bufs=1, space="PSUM"))

            # ---- load constants once
            wt_sb = consts.tile([128, 2, 256], BF16)
            wa_sb = consts.tile([128, 2, 256], BF16)
            w1_sb = consts.tile([128, 2, 512], BF16)
            w2_sb = consts.tile([128, 4, 256], BF16)
            for k in range(2):
                nc.sync.dma_start(out=wt_sb[:, k, :], in_=wt_d[k, :, :])
                nc.sync.dma_start(out=wa_sb[:, k, :], in_=wa_d[k, :, :])
                nc.sync.dma_start(out=w1_sb[:, k, :], in_=w1_d[k, :, :])
            for k in range(4):
                nc.sync.dma_start(out=w2_sb[:, k, :], in_=w2_d[k, :, :])
            band1c = consts.tile([128, 128], BF16, tag="b1c")
            nc.sync.dma_start(out=band1c, in_=b1c_d[:, :])
            band1p = consts.tile([128, 15], BF16, tag="b1p")
            nc.sync.dma_start(out=band1p, in_=b1p_d[:, :])
            band2c = consts.tile([128, 128], BF16, tag="b2c")
            nc.sync.dma_start(out=band2c, in_=b2c_d[:, :])
            ecols = consts.tile([128, 4], BF16, tag="ec")
            nc.sync.dma_start(out=ecols, in_=ec_d[:, :])
            ones_sb = consts.tile([1, 128], BF16, tag="ones")
            nc.sync.dma_start(out=ones_sb, in_=ones_d[:, :])
            browp_sb = consts.tile([1, 3, 512], BF16, tag="browp")
            nc.sync.dma_start(out=browp_sb, in_=browp_d[:, :])
            b1_sb = consts.tile([128, 4], F32, tag="b1c2")
            nc.sync.dma_start(out=b1_sb, in_=b1col_d[:, :])
            idb_sb = consts.tile([128, 128], BF16, tag="idb")
            nc.sync.dma_start(out=idb_sb, in_=idb_d[:, :])
            corr_sb = None
            if need_corr:
                corr_sb = consts.tile([15, 256], F32, tag="corr")
                nc.sync.dma_start(out=corr_sb, in_=corr_d[:, :])

            st = {}

            # ---------------- helpers

            def batch_combine(ln, b):
                """Combine grouped bn_stats halves + rsqrt for batch b of LN
                `ln`: s6b [128, 2RB, 6] -> mean/rstd [128, 2RB].

                bn_stats per tile emits (c_even, m_e, c*var_e, c_odd, m_o,
                c*var_o) over the 256-wide feature dim (even/odd split,
                c=128 each):
                  mean = (m_e + m_o)/2
                  var  = (cv_e + cv_o)/256 + ((m_e - m_o)/2)^2
                """
                e = eng(E_BATCH)
                n = 2 * RB
                s6b = st.pop(("s6", ln, b))
                me = s6b[:, :, 1:2].rearrange("p a b -> p (a b)")
                mo = s6b[:, :, 4:5].rearrange("p a b -> p (a b)")
                cve = s6b[:, :, 2:3].rearrange("p a b -> p (a b)")
                cvo = s6b[:, :, 5:6].rearrange("p a b -> p (a b)")
                mb = statp.tile([128, n], F32, tag=f"mb{ln}", name=f"mb{ln}")
                msum = smalls.tile([128, n], F32, tag="c_ms")
                e.tensor_tensor(out=msum, in0=me, in1=mo, op=ALU.add)
                e.tensor_scalar(out=mb, in0=msum, scalar1=0.5, scalar2=None,
                                op0=ALU.mult)
                dm = smalls.tile([128, n], F32, tag="c_dm")
                e.tensor_tensor(out=dm, in0=me, in1=mo, op=ALU.subtract)
                dm2 = smalls.tile([128, n], F32, tag="c_dm2")
                e.tensor_tensor(out=dm2, in0=dm, in1=dm, op=ALU.mult)
                cvs = smalls.tile([128, n], F32, tag="c_cvs")
                e.tensor_tensor(out=cvs, in0=cve, in1=cvo, op=ALU.add)
                var = smalls.tile([128, n], F32, tag="c_var")
                e.tensor_scalar(out=var, in0=cvs, scalar1=1.0 / 256.0,
                                scalar2=None, op0=ALU.mult)
                e.tensor_scalar(out=dm2, in0=dm2, scalar1=0.25, scalar2=None,
                                op0=ALU.mult)
                e.tensor_tensor(out=var, in0=var, in1=dm2, op=ALU.add)
                # rsqrt: quadratic seed (minimax fit of v^-1/2 on [0.3,3.5],
                # 19% max err; int bit-trick opcodes are illegal on GPSIMD)
                # + 2 Newton steps -> <=0.4% worst-case, ~1e-4 in-domain
                rb_t = statp.tile([128, n], F32, tag=f"rb{ln}", name=f"rb{ln}")
                t1 = smalls.tile([128, n], F32, tag="c_t1")
                e.tensor_scalar(out=t1, in0=var, scalar1=0.11963791, scalar2=-0.73880681,
                                op0=ALU.mult, op1=ALU.add)
                t2 = smalls.tile([128, n], F32, tag="c_t2")
                e.tensor_tensor(out=t2, in0=t1, in1=var, op=ALU.mult)
                y0f = smalls.tile([128, n], F32, tag="c_y0f")
                e.tensor_scalar(out=y0f, in0=t2, scalar1=1.69054049, scalar2=None,
                                op0=ALU.add)
                a = smalls.tile([128, n], F32, tag="c_a")
                e.tensor_tensor(out=a, in0=y0f, in1=y0f, op=ALU.mult)
                bb = smalls.tile([128, n], F32, tag="c_b")
                e.tensor_tensor(out=bb, in0=var, in1=a, op=ALU.mult)
                cc = smalls.tile([128, n], F32, tag="c_c")
                e.tensor_scalar(out=cc, in0=bb, scalar1=-0.5, scalar2=1.5,
                                op0=ALU.mult, op1=ALU.add)
                y1 = smalls.tile([128, n], F32, tag="c_y1")
                e.tensor_tensor(out=y1, in0=y0f, in1=cc, op=ALU.mult)
                e.tensor_tensor(out=a, in0=y1, in1=y1, op=ALU.mult)
                e.tensor_tensor(out=bb, in0=var, in1=a, op=ALU.mult)
                e.tensor_scalar(out=cc, in0=bb, scalar1=-0.5, scalar2=1.5,
                                op0=ALU.mult, op1=ALU.add)
                e.tensor_tensor(out=rb_t, in0=y1, in1=cc, op=ALU.mult)
                st[("mb", ln, b)] = mb
                st[("rb", ln, b)] = rb_t

            def stats_grouped(ln, p, src_ap):
                """Per-tile bn_stats for pair p of LN ln into the batch buffer
                (the [128,2,256] grouped form coalesces in AP lowering and
                mixes the two tiles — must stay per-tile); batch at RB end."""
                b = p // RB
                if p % RB == 0:
                    st[("s6", ln, b)] = statp.tile([128, 2 * RB, 6], F32,
                                                   tag=f"s6_{ln}", name=f"s6_{ln}")
                s6b = st[("s6", ln, b)]
                j = p % RB
                for t in range(2):
                    nc.vector.bn_stats(s6b[:, 2 * j + t, :], src_ap[:, t, :])
                if p % RB == RB - 1:
                    batch_combine(ln, b)

            def normalize(ename, ln, p, src, dst):
                """dst[:,t,:] = (src[:,t,:] - mean)*rstd for both tiles."""
                e = eng(ename)
                b = p // RB
                mb = st[("mb", ln, b)]
                rb_t = st[("rb", ln, b)]
                for t in range(2):
                    j = 2 * (p % RB) + t
                    e.tensor_scalar(out=dst[:, t, :], in0=src[:, t, :],
                                    scalar1=mb[:, j:j + 1], scalar2=rb_t[:, j:j + 1],
                                    op0=ALU.subtract, op1=ALU.mult)

            # ---------------- stage functions (pair-granular) ----------------

            def sA(p):
                """DMA x pair p; grouped LN1 stats."""
                xp = xpool.tile([128, 2, 256], F32R, tag="x")
                lo = p * 256
                nc.sync.dma_start(
                    out=xp, in_=x_d[lo:lo + 256, :].rearrange("(a p) d -> p a d", a=2))
                st[("x", p)] = xp
                stats_grouped(1, p, xp.bitcast(F32))

            def sA2(p):
                """LN1 normalize -> xn1 bf16."""
                xp = st[("x", p)]
                xn = xn1p.tile([128, 2, 256], BF16, tag="xn1")
                normalize(E_NORM1, 1, p, xp.bitcast(F32), xn)
                st[("xn1", p)] = xn

            def sB(p):
                """Temporal band matmuls for pair p -> agg1 PSUM; evac ACT."""
                xn = st[("xn1", p)]
                xnm = st.get(("xn1", p - 1))
                agg = pps.tile([128, 2, 2, 128], F32, tag="aggT", bufs=2, name="agg1")
                for t in range(2):
                    g = 2 * p + t
                    prev = xn[:, 0, :] if t == 1 else (xnm[:, 1, :] if xnm is not None else None)
                    for h in range(2):
                        hs = slice(h * 128, (h + 1) * 128)
                        nc.tensor.matmul(agg[:, t, h, :], xn[:, t, hs], band1c,
                                         start=True, stop=(g == 0))
                        if g > 0:
                            nc.tensor.matmul(agg[:, t, h, 0:15], prev[:, hs], band1p,
                                             start=False, stop=True)
                a1 = a1p.tile([128, 2, 2, 128], BF16, tag="a1sb")
                nc.scalar.activation(a1, agg, AF.Copy)
                st[("a1sb", p)] = a1
                st.pop(("xn1", p - 1), None)

            def sC(p):
                """proj1 + bias -> att1 PSUM."""
                a1 = st.pop(("a1sb", p))
                att = pps.tile([128, 2, 256], F32, tag="att1", bufs=1, name="att1")
                for t in range(2):
                    for h in range(2):
                        nc.tensor.matmul(att[:, t, :], a1[:, t, h, :], wt_sb[:, h, :],
                                         start=(t == 0 and h == 0), stop=False)
                attf = att.rearrange("p a d -> p (a d)")
                nc.tensor.matmul(attf, ones_sb, browp_sb[:, 0, :], start=False, stop=True)
                xp = st.pop(("x", p))
                x1 = x1p.tile([128, 2, 256], BF16, tag="x1")
                nc.vector.tensor_tensor(out=x1, in0=att, in1=xp.bitcast(F32),
                                        op=ALU.add)
                if need_corr and p == 0:
                    nc.vector.tensor_tensor(out=x1[0:15, 0, :], in0=x1[0:15, 0, :],
                                            in1=corr_sb, op=ALU.add)
                st[("x1", p)] = x1

            def sD(p):
                """LN2 stats on x1 (grouped)."""
                stats_grouped(2, p, st[("x1", p)])

            def sD2(p):
                """LN2 normalize -> xn2 bf16."""
                x1 = st[("x1", p)]
                xn = xn2p.tile([128, 2, 256], BF16, tag="xn2")
                normalize(E_NORM23, 2, p, x1, xn)
                st[("xn2", p)] = xn

            def sE(p):
                """Neighbor band for pair p (needs xn2 of pairs p-1, p, p+1)."""
                xn = st[("xn2", p)]
                xnm = st.get(("xn2", p - 1))
                xnp = st.get(("xn2", p + 1))
                agg = pps.tile([128, 2, 2, 128], F32, tag="aggT", bufs=2, name="agg2")
                for t in range(2):
                    g = 2 * p + t
                    prev = xn[:, 0, :] if t == 1 else (xnm[:, 1, :] if xnm is not None else None)
                    nxt = xn[:, 1, :] if t == 0 else (xnp[:, 0, :] if xnp is not None else None)
                    for h in range(2):
                        hs = slice(h * 128, (h + 1) * 128)
                        nc.tensor.matmul(agg[:, t, h, :], xn[:, t, hs], band2c,
                                         start=True, stop=False)
                        if g > 0:
                            nc.tensor.matmul(agg[:, t, h, 0:1], prev[:, hs],
                                             ecols[:, 0:1], start=False, stop=False)
                        else:
                            nc.tensor.matmul(agg[:, t, h, 0:1], xn[:, t, hs],
                                             ecols[:, 1:2], start=False, stop=False)
                        if g < NT - 1:
                            nc.tensor.matmul(agg[:, t, h, 127:128], nxt[:, hs],
                                             ecols[:, 2:3], start=False, stop=True)
                        else:
                            nc.tensor.matmul(agg[:, t, h, 127:128], xn[:, t, hs],
                                             ecols[:, 3:4], start=False, stop=True)
                a2 = a2p.tile([128, 2, 2, 128], BF16, tag="a2sb")
                nc.scalar.activation(a2, agg, AF.Copy)
                st[("a2sb", p)] = a2
                if p - 2 >= 0:
                    st.pop(("xn2", p - 2), None)

            def sF(p):
                """proj2 + bias -> att2."""
                a2 = st.pop(("a2sb", p))
                att = pps.tile([128, 2, 256], F32, tag="att2", bufs=1, name="att2")
                for t in range(2):
                    for h in range(2):
                        nc.tensor.matmul(att[:, t, :], a2[:, t, h, :], wa_sb[:, h, :],
                                         start=(t == 0 and h == 0), stop=False)
                attf = att.rearrange("p a d -> p (a d)")
                nc.tensor.matmul(attf, ones_sb, browp_sb[:, 1, :], start=False, stop=True)
                x1 = st.pop(("x1", p))
                x2 = x2p.tile([128, 2, 256], BF16, tag="x2")
                nc.vector.tensor_tensor(out=x2, in0=att, in1=x1, op=ALU.add)
                st[("x2", p)] = x2

            def sG(p):
                """LN3 stats (grouped)."""
                stats_grouped(3, p, st[("x2", p)])

            def sG2(p):
                """LN3 normalize -> xn3 bf16."""
                x2 = st[("x2", p)]
                xn = xn3p.tile([128, 2, 256], BF16, tag="xn3")
                normalize(E_NORM23, 3, p, x2, xn)
                st[("xn3", p)] = xn

            def sH(p):
                """Transpose xn3 pair -> PSUM [128,(h,t),128]; evac -> xn3T."""
                xn = st.pop(("xn3", p))
                tp = pps.tile([128, 2, 2, 128], BF16, tag="aggT", bufs=2, name="x3t")
                for t in range(2):
                    for h in range(2):
                        nc.tensor.transpose(tp[:, h, t, :], xn[:, t, h * 128:(h + 1) * 128],
                                            idb_sb)
                sbn = p // SBP
                buf = st[("xn3T", sbn)]
                lo = (p % SBP) * 256
                dst = buf[:, :, lo:lo + 256]
                src = tp.rearrange("p h t k -> p h (t k)")
                nc.scalar.activation(dst, src, AF.Copy)

            def ffn1_unit(sbn, q, m):
                """One FFN1 (q,m) unit: 2 matmuls + gelu."""
                xbuf = st[("xn3T", sbn)]
                if ("gbuf", sbn) not in st:
                    st[("gbuf", sbn)] = gelup.tile([128, 4, SBP * 256], BF16,
                                                   tag="gbuf", name="gbuf")
                gbuf = st[("gbuf", sbn)]
                qs = slice(q * 512, (q + 1) * 512)
                gps = pps.tile([128, 512], F32, tag="gps", bufs=2, name="gps")
                ms = slice(m * 128, (m + 1) * 128)
                nc.tensor.matmul(gps, w1_sb[:, 0, ms], xbuf[:, 0, qs],
                                 start=True, stop=False)
                nc.tensor.matmul(gps, w1_sb[:, 1, ms], xbuf[:, 1, qs],
                                 start=False, stop=True)
                nc.scalar.activation(gbuf[:, m, qs], gps, GELU,
                                     bias=b1_sb[:, m:m + 1])

            def sJ(p):
                """FFN2 + bias + x2 residual -> A3; evac out pair f32; DMA."""
                sbn = p // SBP
                gbuf = st[("gbuf", sbn)]
                att = pps.tile([128, 2, 256], F32, tag="att3", bufs=2, name="att3")
                for t in range(2):
                    cs = slice((p % SBP) * 256 + t * 128, (p % SBP) * 256 + (t + 1) * 128)
                    for k in range(4):
                        nc.tensor.matmul(att[:, t, :], gbuf[:, k, cs], w2_sb[:, k, :],
                                         start=(t == 0 and k == 0), stop=False)
                attf = att.rearrange("p a d -> p (a d)")
                nc.tensor.matmul(attf, ones_sb, browp_sb[:, 2, :], start=False, stop=False)
                x2 = st.pop(("x2", p))
                nc.tensor.matmul(attf, idb_sb, x2.rearrange("p a d -> p (a d)"),
                                 start=False, stop=True)
                ot = outp.tile([128, 2, 256], F32, tag="out")
                nc.scalar.activation(ot, att, AF.Copy)
                lo = p * 256
                nc.sync.dma_start(
                    out=out_d[lo:lo + 256, :].rearrange("(a p) d -> p a d", a=2), in_=ot)
                if p % SBP == SBP - 1:
                    st.pop(("gbuf", sbn), None)

            # ---------------- emission: software-pipelined over pairs

            def body():
                st.clear()
                for s_ in range(NSB):
                    st[("xn3T", s_)] = bigp.tile([128, 2, SBP * 256], BF16, tag="xn3T", name="xn3T")
                stages = [(sA, 0), (sA2, 5), (sB, 6), (sC, 7),
                          (sD, 8), (sD2, 13), (sE, 14), (sF, 15),
                          (sG, 16), (sG2, 21), (sH, 22)]
                DH = 22                 # sH delay
                import collections as _c
                jq = _c.deque()
                uq = _c.deque()
                rel = {}

                def emit_unit():
                    if uq:
                        sbn, q, m = uq.popleft()
                        ffn1_unit(sbn, q, m)

                for s_ in range(NP + DH + 12):
                    for fn, d_ in stages:
                        i = s_ - d_
                        if 0 <= i < NP:
                            fn(i)
                        if fn in (sB, sE, sH):
                            emit_unit()
                    # after sH of an odd pair, its q-column of the superblock
                    # is complete: enqueue that column's 4 FFN1 units and allow
                    # the two sJ's once the units have had 2 steps to drain
                    ph = s_ - DH
                    if 0 <= ph < NP and ph % 2 == 1:
                        sbn, q = ph // SBP, (ph % SBP) // 2
                        uq.extend((sbn, q, m) for m in range(4))
                        jq.extend((ph - 1, ph))
                        rel[ph - 1] = s_ + 2
                        rel[ph] = s_ + 2
                    emit_unit()
                    if jq and s_ >= rel[jq[0]]:
                        sJ(jq.popleft())
                    emit_unit()
                while uq:
                    emit_unit()
                while jq:
                    sJ(jq.popleft())

            if repeat > 1:
                with tc.For_i(0, repeat, 1):
                    body()
            else:
                body()

    nc.compile()
    return nc


# ---------------------------------------------------------------- entry

def _run(inputs, repeat=1, n_calls=1, gelu=True):
    import time
    consts, corr, need_corr = _host_consts(inputs)
    nc = build_nc(repeat=repeat, need_corr=need_corr, gelu=gelu)
    x = np.asarray(inputs["x"], np.float32)
    in_maps = []
    for b in range(B):
        m = {"x": np.ascontiguousarray(x[b])}
        for k, v in consts.items():
            m[k] = v
        if need_corr:
            m["corr"] = corr
        in_maps.append(m)
    times = []
    res = None
    for _ in range(n_calls):
        t0 = time.time()
        res = bass_utils.run_bass_kernel_spmd(nc, in_maps, core_ids=list(range(B)))
        times.append(time.time() - t0)
    out = np.stack([res.results[b]["out"] for b in range(B)]).astype(np.float32)
    return out, times


def kernel(**inputs) -> np.ndarray:
    try:
        out, _ = _run(inputs, repeat=1, n_calls=1)
    except Exception:
        # transient device wedges have been observed; one retry
        out, _ = _run(inputs, repeat=1, n_calls=1)
    return out


# revision 18
# speedup vs baseline: 1.7547x; 1.7547x over previous
"""Trainium2 Bass kernel for nn_BandProcessor — v3 (engine-rebalanced).

Structure per 128-token tile (64 tiles/core, batch data-parallel over 8 cores):
  L1: LN1 -> temporal 16-tap causal band (PE band matmul + N=15 spill) ->
      proj (+bias in PSUM) -> x residual added during PSUM evac -> x1
  L2: LN2 -> 3-tap neighbor band (N=1 edge columns) -> proj (+bias) ->
      x1 residual on evac -> x2
  FFN: LN3 -> transpose -> W1+gelu -> W2 (+bias +x2 residual matmul) -> out

Engine assignment (v2 bottlenecked DVE at ~95%; HW-microbenched per-op
costs drove this rebalance):
  - DVE: per-tile bn_stats into RB=8 batch buffers, batched mean/var
    combine + polynomial-seed rsqrt (replaces all 192 bn_aggr calls),
    all three LN normalizes, x1 = att1+x residual evac (tensor_tensor)
  - ACT: a1/a2/xn3T/out/x2 evacs (Copy), gelu (bias rides the activation)
  - PE: band/proj/FFN matmuls + ones-row bias matmuls + x1/x2 residual
    identity matmuls for the ACT-evac'd layers
  - GPSIMD unused: measured ~2x DVE with large per-op overhead on HW
  - PSUM 8 banks: aggT 2 + att1 1 + att2 1 + att3 2 + gps 2; gps/att3
    double-buffering de-serializes the FFN tail
  - FFN1 units enqueue per completed 512-token q-column (not per
    superblock) so FFN2/out drain overlaps the main pipeline
"""

import numpy as np
import ml_dtypes

import concourse.bacc as bacc
import concourse.mybir as mybir
from concourse.tile import TileContext
from concourse import bass_utils

B, T, D = 8, 8192, 256
H = 16
DECAY = 0.9
EPS = 1e-5
NT = T // 128           # 64 token tiles per core
NP = NT // 2            # 32 pairs
SBP = 8                 # pairs per superblock (FFN batching) -> 2048 tokens
NSB = NP // SBP         # 4 superblocks
RB = 4                  # pairs per stats batch (8 tiles)

F32 = mybir.dt.float32
F32R = mybir.dt.float32r
BF16 = mybir.dt.bfloat16
I32 = mybir.dt.int32

AF = mybir.ActivationFunctionType
ALU = mybir.AluOpType

# engine knobs (quick rebalance without restructuring).
# NOTE: GPSIMD measured ~2x slower with large per-op overhead on HW —
# the cost model's 61ns Pool seq overhead is wildly optimistic. Avoid.
E_NORM1 = "vector"
E_NORM23 = "vector"
E_BATCH = "vector"


# ---------------------------------------------------------------- host prep

def _host_consts(inp, gelu_ok=True):
    """Fold LN gains + value/out projections into single matrices (f64)."""
    g1, b1_ = inp["n1_g"].astype(np.float64), inp["n1_b"].astype(np.float64)
    g2, b2_ = inp["n2_g"].astype(np.float64), inp["n2_b"].astype(np.float64)
    g3, b3_ = inp["n3_g"].astype(np.float64), inp["n3_b"].astype(np.float64)
    t_Wv, t_bv = inp["t_Wv"].astype(np.float64), inp["t_bv"].astype(np.float64)
    t_Wo, t_bo = inp["t_Wo"].astype(np.float64), inp["t_bo"].astype(np.float64)
    a_Wv, a_bv = inp["a_Wv"].astype(np.float64), inp["a_bv"].astype(np.float64)
    a_Wo, a_bo = inp["a_Wo"].astype(np.float64), inp["a_bo"].astype(np.float64)
    f_W1, f_b1 = inp["f_W1"].astype(np.float64), inp["f_b1"].astype(np.float64)
    f_W2, f_b2 = inp["f_W2"].astype(np.float64), inp["f_b2"].astype(np.float64)

    WtWo = t_Wv @ t_Wo
    WaWo = a_Wv @ a_Wo
    Wt_eff = g1[:, None] * WtWo                                # [D, D]
    bt_eff = b1_ @ WtWo + t_bv @ t_Wo + t_bo
    Wa_eff = g2[:, None] * WaWo
    ba_eff = b2_ @ WaWo + a_bv @ a_Wo + a_bo
    W1_eff = g3[:, None] * f_W1                                # [D, 2D]
    b1_eff = b3_ @ f_W1 + f_b1                                 # [2D]
    W2 = f_W2
    b2v = f_b2

    # temporal weights: reference tw[j] applies to h_pad[j:j+T];
    # lag d = H-1-j  ->  w_lag[d] = tw[H-1-d]
    tw = DECAY ** np.arange(H, dtype=np.float64)
    tw = tw / tw.sum()
    w_lag = tw[::-1].copy()

    band1c = np.zeros((128, 128), np.float64)
    for ti in range(128):
        for to in range(ti, min(128, ti + H)):
            band1c[ti, to] = w_lag[to - ti]
    # spill into next tile: cols 0..14 get lags 1..15 from prev tile's
    # partitions 113..127
    band1p15 = np.zeros((128, 15), np.float64)
    for p in range(113, 128):
        for to in range(0, p - 112):
            band1p15[p, to] = w_lag[to + 128 - p]
    band2c = np.zeros((128, 128), np.float64)
    for ti in range(128):
        for to in range(max(0, ti - 1), min(128, ti + 2)):
            band2c[ti, to] = 1.0 / 3.0
    ep_col = np.zeros((128, 1), np.float64); ep_col[127, 0] = 1.0 / 3.0
    ep0_col = np.zeros((128, 1), np.float64); ep0_col[0, 0] = 1.0 / 3.0
    en_col = np.zeros((128, 1), np.float64); en_col[0, 0] = 1.0 / 3.0
    en63_col = np.zeros((128, 1), np.float64); en63_col[127, 0] = 1.0 / 3.0

    bf = lambda a: np.ascontiguousarray(a).astype(ml_dtypes.bfloat16)
    f32 = lambda a: np.ascontiguousarray(a).astype(np.float32)

    # first-tile correction for the temporal zero-pad of the LN bias term
    c_t = np.cumsum(w_lag)[:H - 1]
    corr = f32((c_t - 1.0)[:, None] * (b1_ @ WtWo)[None, :])

    consts = {
        "wt": bf(np.stack([Wt_eff[0:128], Wt_eff[128:256]])),       # [2,128,256]
        "wa": bf(np.stack([Wa_eff[0:128], Wa_eff[128:256]])),
        "w1": bf(np.stack([W1_eff[0:128], W1_eff[128:256]])),       # [2,128,512]
        "w2": bf(np.stack([W2[k * 128:(k + 1) * 128] for k in range(4)])),
        "band1c": bf(band1c), "band1p15": bf(band1p15),
        "band2c": bf(band2c),
        "ecols": bf(np.concatenate([ep_col, ep0_col, en_col, en63_col], axis=1)),
        "ones_r": bf(np.ones((1, 128))),
        # biases duplicated for pair-wide N=512 matmuls
        "browp": bf(np.stack([np.tile(bt_eff, 2), np.tile(ba_eff, 2),
                              np.tile(b2v, 2)])),                   # [3,512]
        "b1col": f32(b1_eff.reshape(4, 128).T),                     # [128,4]
        "identb": bf(np.eye(128)),
    }
    need_corr = bool(np.abs(corr).max() > 0)
    return consts, corr, need_corr


# ---------------------------------------------------------------- bass build

def build_nc(repeat=1, need_corr=False, gelu=True):
    nc = bacc.Bacc("TRN2", target_bir_lowering=False, debug=False, num_devices=8)
    GELU = AF.Gelu if gelu else AF.Identity

    x_d = nc.dram_tensor("x", (T, D), F32R, kind="ExternalInput")
    out_d = nc.dram_tensor("out", (T, D), F32, kind="ExternalOutput")
    wt_d = nc.dram_tensor("wt", (2, 128, 256), BF16, kind="ExternalInput")
    wa_d = nc.dram_tensor("wa", (2, 128, 256), BF16, kind="ExternalInput")
    w1_d = nc.dram_tensor("w1", (2, 128, 512), BF16, kind="ExternalInput")
    w2_d = nc.dram_tensor("w2", (4, 128, 256), BF16, kind="ExternalInput")
    b1c_d = nc.dram_tensor("band1c", (128, 128), BF16, kind="ExternalInput")
    b1p_d = nc.dram_tensor("band1p15", (128, 15), BF16, kind="ExternalInput")
    b2c_d = nc.dram_tensor("band2c", (128, 128), BF16, kind="ExternalInput")
    ec_d = nc.dram_tensor("ecols", (128, 4), BF16, kind="ExternalInput")
    ones_d = nc.dram_tensor("ones_r", (1, 128), BF16, kind="ExternalInput")
    browp_d = nc.dram_tensor("browp", (3, 512), BF16, kind="ExternalInput")
    b1col_d = nc.dram_tensor("b1col", (128, 4), F32, kind="ExternalInput")
    idb_d = nc.dram_tensor("identb", (128, 128), BF16, kind="ExternalInput")
    corr_d = nc.dram_tensor("corr", (15, 256), F32, kind="ExternalInput") if need_corr else None

    def eng(name):
        return {"vector": nc.vector, "gpsimd": nc.gpsimd}[name]

    with TileContext(nc) as tc:
        import contextlib
        ctx = contextlib.ExitStack()
        with ctx:
            consts = ctx.enter_context(tc.tile_pool(name="consts", bufs=1))
            xpool = ctx.enter_context(tc.tile_pool(name="xpool", bufs=14))
            xn1p = ctx.enter_context(tc.tile_pool(name="xn1p", bufs=11))
            a1p = ctx.enter_context(tc.tile_pool(name="a1p", bufs=3))
            x1p = ctx.enter_context(tc.tile_pool(name="x1p", bufs=15))
            xn2p = ctx.enter_context(tc.tile_pool(name="xn2p", bufs=11))
            a2p = ctx.enter_context(tc.tile_pool(name="a2p", bufs=3))
            x2p = ctx.enter_context(tc.tile_pool(name="x2p", bufs=17))
            xn3p = ctx.enter_context(tc.tile_pool(name="xn3p", bufs=11))
            statp = ctx.enter_context(tc.tile_pool(name="statp", bufs=2))
            bigp = ctx.enter_context(tc.tile_pool(name="bigp", bufs=2))
            gelup = ctx.enter_context(tc.tile_pool(name="gelup", bufs=2))
            outp = ctx.enter_context(tc.tile_pool(name="outp", bufs=3))
            smalls = ctx.enter_context(tc.tile_pool(name="smalls", bufs=2))
            # PSUM: 8 banks total, every tile 1 full bank; per-tag bufs:
            # aggT 2 + att1 1 + att2 1 + att3 2 + gps 2 = 8
            pps = ctx.enter_context(tc.tile_pool(name="pps", bufs=1, space="PSUM"))

            # ---- load constants once
            wt_sb = consts.tile([128, 2, 256], BF16)
            wa_sb = consts.tile([128, 2, 256], BF16)
            w1_sb = consts.tile([128, 2, 512], BF16)
            w2_sb = consts.tile([128, 4, 256], BF16)
            for k in range(2):
                nc.sync.dma_start(out=wt_sb[:, k, :], in_=wt_d[k, :, :])
                nc.sync.dma_start(out=wa_sb[:, k, :], in_=wa_d[k, :, :])
                nc.sync.dma_start(out=w1_sb[:, k, :], in_=w1_d[k, :, :])
            for k in range(4):
                nc.sync.dma_start(out=w2_sb[:, k, :], in_=w2_d[k, :, :])
            band1c = consts.tile([128, 128], BF16, tag="b1c")
            nc.sync.dma_start(out=band1c, in_=b1c_d[:, :])
            band1p = consts.tile([128, 15], BF16, tag="b1p")
            nc.sync.dma_start(out=band1p, in_=b1p_d[:, :])
            band2c = consts.tile([128, 128], BF16, tag="b2c")
            nc.sync.dma_start(out=band2c, in_=b2c_d[:, :])
            ecols = consts.tile([128, 4], BF16, tag="ec")
            nc.sync.dma_start(out=ecols, in_=ec_d[:, :])
            ones_sb = consts.tile([1, 128], BF16, tag="ones")
            nc.sync.dma_start(out=ones_sb, in_=ones_d[:, :])
            browp_sb = consts.tile([1, 3, 512], BF16, tag="browp")
            nc.sync.dma_start(out=browp_sb, in_=browp_d[:, :])
            b1_sb = consts.tile([128, 4], F32, tag="b1c2")
            nc.sync.dma_start(out=b1_sb, in_=b1col_d[:, :])
            idb_sb = consts.tile([128, 128], BF16, tag="idb")
            nc.sync.dma_start(out=idb_sb, in_=idb_d[:, :])
            corr_sb = None
            if need_corr:
                corr_sb = consts.tile([15, 256], F32, tag="corr")
                nc.sync.dma_start(out=corr_sb, in_=corr_d[:, :])

            st = {}

            # ---------------- helpers

            def batch_combine(ln, b):
                """Combine grouped bn_stats halves + rsqrt for batch b of LN
                `ln`: s6b [128, 2RB, 6] -> mean/rstd [128, 2RB].

                bn_stats per tile emits (c_even, m_e, c*var_e, c_odd, m_o,
                c*var_o) over the 256-wide feature dim (even/odd split,
                c=128 each):
                  mean = (m_e + m_o)/2
                  var  = (cv_e + cv_o)/256 + ((m_e - m_o)/2)^2
                """
                e = eng(E_BATCH)
                n = 2 * RB
                s6b = st.pop(("s6", ln, b))
                me = s6b[:, :, 1:2].rearrange("p a b -> p (a b)")
                mo = s6b[:, :, 4:5].rearrange("p a b -> p (a b)")
                cve = s6b[:, :, 2:3].rearrange("p a b -> p (a b)")
                cvo = s6b[:, :, 5:6].rearrange("p a b -> p (a b)")
                mb = statp.tile([128, n], F32, tag=f"mb{ln}", name=f"mb{ln}")
                msum = smalls.tile([128, n], F32, tag="c_ms")
                e.tensor_tensor(out=msum, in0=me, in1=mo, op=ALU.add)
                e.tensor_scalar(out=mb, in0=msum, scalar1=0.5, scalar2=None,
                                op0=ALU.mult)
                dm = smalls.tile([128, n], F32, tag="c_dm")
                e.tensor_tensor(out=dm, in0=me, in1=mo, op=ALU.subtract)
                dm2 = smalls.tile([128, n], F32, tag="c_dm2")
                e.tensor_tensor(out=dm2, in0=dm, in1=dm, op=ALU.mult)
                cvs = smalls.tile([128, n], F32, tag="c_cvs")
                e.tensor_tensor(out=cvs, in0=cve, in1=cvo, op=ALU.add)
                var = smalls.tile([128, n], F32, tag="c_var")
                e.tensor_scalar(out=var, in0=cvs, scalar1=1.0 / 256.0,
                                scalar2=None, op0=ALU.mult)
                e.tensor_scalar(out=dm2, in0=dm2, scalar1=0.25, scalar2=None,
                                op0=ALU.mult)
                e.tensor_tensor(out=var, in0=var, in1=dm2, op=ALU.add)
                # rsqrt: quadratic seed (minimax fit of v^-1/2 on [0.3,3.5],
                # 19% max err; int bit-trick opcodes are illegal on GPSIMD)
                # + 2 Newton steps -> <=0.4% worst-case, ~1e-4 in-domain
                rb_t = statp.tile([128, n], F32, tag=f"rb{ln}", name=f"rb{ln}")
                t1 = smalls.tile([128, n], F32, tag="c_t1")
                e.tensor_scalar(out=t1, in0=var, scalar1=0.11963791, scalar2=-0.73880681,
                                op0=ALU.mult, op1=ALU.add)
                t2 = smalls.tile([128, n], F32, tag="c_t2")
                e.tensor_tensor(out=t2, in0=t1, in1=var, op=ALU.mult)
                y0f = smalls.tile([128, n], F32, tag="c_y0f")
                e.tensor_scalar(out=y0f, in0=t2, scalar1=1.69054049, scalar2=None,
                                op0=ALU.add)
                a = smalls.tile([128, n], F32, tag="c_a")
                e.tensor_tensor(out=a, in0=y0f, in1=y0f, op=ALU.mult)
                bb = smalls.tile([128, n], F32, tag="c_b")
                e.tensor_tensor(out=bb, in0=var, in1=a, op=ALU.mult)
                cc = smalls.tile([128, n], F32, tag="c_c")
                e.tensor_scalar(out=cc, in0=bb, scalar1=-0.5, scalar2=1.5,
                                op0=ALU.mult, op1=ALU.add)
                y1 = smalls.tile([128, n], F32, tag="c_y1")
                e.tensor_tensor(out=y1, in0=y0f, in1=cc, op=ALU.mult)
                e.tensor_tensor(out=a, in0=y1, in1=y1, op=ALU.mult)
                e.tensor_tensor(out=bb, in0=var, in1=a, op=ALU.mult)
                e.tensor_scalar(out=cc, in0=bb, scalar1=-0.5, scalar2=1.5,
                                op0=ALU.mult, op1=ALU.add)
                e.tensor_tensor(out=rb_t, in0=y1, in1=cc, op=ALU.mult)
                st[("mb", ln, b)] = mb
                st[("rb", ln, b)] = rb_t

            def stats_grouped(ln, p, src_ap):
                """Per-tile bn_stats for pair p of LN ln into the batch buffer
                (the [128,2,256] grouped form coalesces in AP lowering and
                mixes the two tiles — must stay per-tile); batch at RB end."""
                b = p // RB
                if p % RB == 0:
                    st[("s6", ln, b)] = statp.tile([128, 2 * RB, 6], F32,
                                                   tag=f"s6_{ln}", name=f"s6_{ln}")
                s6b = st[("s6", ln, b)]
                j = p % RB
                for t in range(2):
                    nc.vector.bn_stats(s6b[:, 2 * j + t, :], src_ap[:, t, :])
                if p % RB == RB - 1:
                    batch_combine(ln, b)

            def normalize(ename, ln, p, src, dst):
                """dst[:,t,:] = (src[:,t,:] - mean)*rstd for both tiles."""
                e = eng(ename)
                b = p // RB
                mb = st[("mb", ln, b)]
                rb_t = st[("rb", ln, b)]
                for t in range(2):
                    j = 2 * (p % RB) + t
                    e.tensor_scalar(out=dst[:, t, :], in0=src[:, t, :],
                                    scalar1=mb[:, j:j + 1], scalar2=rb_t[:, j:j + 1],
                                    op0=ALU.subtract, op1=ALU.mult)

            # ---------------- stage functions (pair-granular) ----------------

            def sA(p):
                """DMA x pair p; grouped LN1 stats."""
                xp = xpool.tile([128, 2, 256], F32R, tag="x")
                lo = p * 256
                nc.sync.dma_start(
                    out=xp, in_=x_d[lo:lo + 256, :].rearrange("(a p) d -> p a d", a=2))
                st[("x", p)] = xp
                stats_grouped(1, p, xp.bitcast(F32))

            def sA2(p):
                """LN1 normalize -> xn1 bf16."""
                xp = st[("x", p)]
                xn = xn1p.tile([128, 2, 256], BF16, tag="xn1")
                normalize(E_NORM1, 1, p, xp.bitcast(F32), xn)
                st[("xn1", p)] = xn

            def sB(p):
                """Temporal band matmuls for pair p -> agg1 PSUM; evac ACT."""
                xn = st[("xn1", p)]
                xnm = st.get(("xn1", p - 1))
                agg = pps.tile([128, 2, 2, 128], F32, tag="aggT", bufs=2, name="agg1")
                for t in range(2):
                    g = 2 * p + t
                    prev = xn[:, 0, :] if t == 1 else (xnm[:, 1, :] if xnm is not None else None)
                    for h in range(2):
                        hs = slice(h * 128, (h + 1) * 128)
                        nc.tensor.matmul(agg[:, t, h, :], xn[:, t, hs], band1c,
                                         start=True, stop=(g == 0))
                        if g > 0:
                            nc.tensor.matmul(agg[:, t, h, 0:15], prev[:, hs], band1p,
                                             start=False, stop=True)
                a1 = a1p.tile([128, 2, 2, 128], BF16, tag="a1sb")
                nc.scalar.activation(a1, agg, AF.Copy)
                st[("a1sb", p)] = a1
                st.pop(("xn1", p - 1), None)

            def sC(p):
                """proj1 + bias -> att1 PSUM."""
                a1 = st.pop(("a1sb", p))
                att = pps.tile([128, 2, 256], F32, tag="att1", bufs=1, name="att1")
                for t in range(2):
                    for h in range(2):
                        nc.tensor.matmul(att[:, t, :], a1[:, t, h, :], wt_sb[:, h, :],
                                         start=(t == 0 and h == 0), stop=False)
                attf = att.rearrange("p a d -> p (a d)")
                nc.tensor.matmul(attf, ones_sb, browp_sb[:, 0, :], start=False, stop=True)
                xp = st.pop(("x", p))
                x1 = x1p.tile([128, 2, 256], BF16, tag="x1")
                nc.vector.tensor_tensor(out=x1, in0=att, in1=xp.bitcast(F32),
                                        op=ALU.add)
                if need_corr and p == 0:
                    nc.vector.tensor_tensor(out=x1[0:15, 0, :], in0=x1[0:15, 0, :],
                                            in1=corr_sb, op=ALU.add)
                st[("x1", p)] = x1

            def sD(p):
                """LN2 stats on x1 (grouped)."""
                stats_grouped(2, p, st[("x1", p)])

            def sD2(p):
                """LN2 normalize -> xn2 bf16."""
                x1 = st[("x1", p)]
                xn = xn2p.tile([128, 2, 256], BF16, tag="xn2")
                normalize(E_NORM23, 2, p, x1, xn)
                st[("xn2", p)] = xn

            def sE(p):
                """Neighbor band for pair p (needs xn2 of pairs p-1, p, p+1)."""
                xn = st[("xn2", p)]
                xnm = st.get(("xn2", p - 1))
                xnp = st.get(("xn2", p + 1))
                agg = pps.tile([128, 2, 2, 128], F32, tag="aggT", bufs=2, name="agg2")
                for t in range(2):
                    g = 2 * p + t
                    prev = xn[:, 0, :] if t == 1 else (xnm[:, 1, :] if xnm is not None else None)
                    nxt = xn[:, 1, :] if t == 0 else (xnp[:, 0, :] if xnp is not None else None)
                    for h in range(2):
                        hs = slice(h * 128, (h + 1) * 128)
                        nc.tensor.matmul(agg[:, t, h, :], xn[:, t, hs], band2c,
                                         start=True, stop=False)
                        if g > 0:
                            nc.tensor.matmul(agg[:, t, h, 0:1], prev[:, hs],
                                             ecols[:, 0:1], start=False, stop=False)
                        else:
                            nc.tensor.matmul(agg[:, t, h, 0:1], xn[:, t, hs],
                                             ecols[:, 1:2], start=False, stop=False)
                        if g < NT - 1:
                            nc.tensor.matmul(agg[:, t, h, 127:128], nxt[:, hs],
                                             ecols[:, 2:3], start=False, stop=True)
                        else:
                            nc.tensor.matmul(agg[:, t, h, 127:128], xn[:, t, hs],
                                             ecols[:, 3:4], start=False, stop=True)
                a2 = a2p.tile([128, 2, 2, 128], BF16, tag="a2sb")
                nc.scalar.activation(a2, agg, AF.Copy)
                st[("a2sb", p)] = a2
                if p - 2 >= 0:
                    st.pop(("xn2", p - 2), None)

            def sF(p):
                """proj2 + bias -> att2."""
                a2 = st.pop(("a2sb", p))
                att = pps.tile([128, 2, 256], F32, tag="att2", bufs=1, name="att2")
                for t in range(2):
                    for h in range(2):
                        nc.tensor.matmul(att[:, t, :], a2[:, t, h, :], wa_sb[:, h, :],
                                         start=(t == 0 and h == 0), stop=False)
                attf = att.rearrange("p a d -> p (a d)")
                nc.tensor.matmul(attf, ones_sb, browp_sb[:, 1, :], start=False, stop=False)
                x1 = st.pop(("x1", p))
                nc.tensor.matmul(attf, idb_sb, x1.rearrange("p a d -> p (a d)"),
                                 start=False, stop=True)
                x2 = x2p.tile([128, 2, 256], BF16, tag="x2")
                nc.scalar.activation(x2, att, AF.Copy)
                st[("x2", p)] = x2

            def sG(p):
                """LN3 stats (grouped)."""
                stats_grouped(3, p, st[("x2", p)])

            def sG2(p):
                """LN3 normalize -> xn3 bf16."""
                x2 = st[("x2", p)]
                xn = xn3p.tile([128, 2, 256], BF16, tag="xn3")
                normalize(E_NORM23, 3, p, x2, xn)
                st[("xn3", p)] = xn

            def sH(p):
                """Transpose xn3 pair -> PSUM [128,(h,t),128]; evac -> xn3T."""
                xn = st.pop(("xn3", p))
                tp = pps.tile([128, 2, 2, 128], BF16, tag="aggT", bufs=2, name="x3t")
                for t in range(2):
                    for h in range(2):
                        nc.tensor.transpose(tp[:, h, t, :], xn[:, t, h * 128:(h + 1) * 128],
                                            idb_sb)
                sbn = p // SBP
                buf = st[("xn3T", sbn)]
                lo = (p % SBP) * 256
                dst = buf[:, :, lo:lo + 256]
                srcv = tp.rearrange("p h t k -> p h (t k)")
                nc.scalar.activation(dst, srcv, AF.Copy)

            def ffn1_unit(sbn, q, m):
                """One FFN1 (q,m) unit: 2 matmuls + gelu."""
                xbuf = st[("xn3T", sbn)]
                if ("gbuf", sbn) not in st:
                    st[("gbuf", sbn)] = gelup.tile([128, 4, SBP * 256], BF16,
                                                   tag="gbuf", name="gbuf")
                gbuf = st[("gbuf", sbn)]
                qs = slice(q * 512, (q + 1) * 512)
                gps = pps.tile([128, 512], F32, tag="gps", bufs=2, name="gps")
                ms = slice(m * 128, (m + 1) * 128)
                nc.tensor.matmul(gps, w1_sb[:, 0, ms], xbuf[:, 0, qs],
                                 start=True, stop=False)
                nc.tensor.matmul(gps, w1_sb[:, 1, ms], xbuf[:, 1, qs],
                                 start=False, stop=True)
                nc.scalar.activation(gbuf[:, m, qs], gps, GELU,
                                     bias=b1_sb[:, m:m + 1])

            def sJ(p):
                """FFN2 + bias + x2 residual -> A3; evac out pair f32; DMA."""
                sbn = p // SBP
                gbuf = st[("gbuf", sbn)]
                att = pps.tile([128, 2, 256], F32, tag="att3", bufs=2, name="att3")
                for t in range(2):
                    cs = slice((p % SBP) * 256 + t * 128, (p % SBP) * 256 + (t + 1) * 128)
                    for k in range(4):
                        nc.tensor.matmul(att[:, t, :], gbuf[:, k, cs], w2_sb[:, k, :],
                                         start=(t == 0 and k == 0), stop=False)
                attf = att.rearrange("p a d -> p (a d)")
                nc.tensor.matmul(attf, ones_sb, browp_sb[:, 2, :], start=False, stop=False)
                x2 = st.pop(("x2", p))
                nc.tensor.matmul(attf, idb_sb, x2.rearrange("p a d -> p (a d)"),
                                 start=False, stop=True)
                ot = outp.tile([128, 2, 256], F32, tag="out")
                nc.scalar.activation(ot, att, AF.Copy)
                lo = p * 256
                nc.sync.dma_start(
                    out=out_d[lo:lo + 256, :].rearrange("(a p) d -> p a d", a=2), in_=ot)
                if p % SBP == SBP - 1:
                    st.pop(("gbuf", sbn), None)

            # ---------------- emission: software-pipelined over pairs

            def body():
                st.clear()
                for s_ in range(NSB):
                    st[("xn3T", s_)] = bigp.tile([128, 2, SBP * 256], BF16, tag="xn3T", name="xn3T")
                stages = [(sA, 0), (sA2, 5), (sB, 6), (sC, 7),
                          (sD, 8), (sD2, 13), (sE, 14), (sF, 15),
                          (sG, 16), (sG2, 21), (sH, 22)]
                DH = 22                 # sH delay
                import collections as _c
                jq = _c.deque()
                uq = _c.deque()
                rel = {}

                def emit_unit():
                    if uq:
                        sbn, q, m = uq.popleft()
                        ffn1_unit(sbn, q, m)

                for s_ in range(NP + DH + 12):
                    for fn, d_ in stages:
                        i = s_ - d_
                        if 0 <= i < NP:
                            fn(i)
                        if fn in (sB, sE, sH):
                            emit_unit()
                    # after sH of an odd pair, its q-column of the superblock
                    # is complete: enqueue that column's 4 FFN1 units and allow
                    # the two sJ's once the units have had 2 steps to drain
                    ph = s_ - DH
                    if 0 <= ph < NP and ph % 2 == 1:
                        sbn, q = ph // SBP, (ph % SBP) // 2
                        uq.extend((sbn, q, m) for m in range(4))
                        jq.extend((ph - 1, ph))
                        rel[ph - 1] = s_ + 2
                        rel[ph] = s_ + 2
                    emit_unit()
                    if jq and s_ >= rel[jq[0]]:
                        sJ(jq.popleft())
                    emit_unit()
                while uq:
                    emit_unit()
                while jq:
                    sJ(jq.popleft())

            if repeat > 1:
                with tc.For_i(0, repeat, 1):
                    body()
            else:
                body()

    nc.compile()
    return nc


# ---------------------------------------------------------------- entry

def _run(inputs, repeat=1, n_calls=1, gelu=True):
    import time
    consts, corr, need_corr = _host_consts(inputs)
    nc = build_nc(repeat=repeat, need_corr=need_corr, gelu=gelu)
    x = np.asarray(inputs["x"], np.float32)
    in_maps = []
    for b in range(B):
        m = {"x": np.ascontiguousarray(x[b])}
        for k, v in consts.items():
            m[k] = v
        if need_corr:
            m["corr"] = corr
        in_maps.append(m)
    times = []
    res = None
    for _ in range(n_calls):
        t0 = time.time()
        res = bass_utils.run_bass_kernel_spmd(nc, in_maps, core_ids=list(range(B)))
        times.append(time.time() - t0)
    out = np.stack([res.results[b]["out"] for b in range(B)]).astype(np.float32)
    return out, times


def kernel(**inputs) -> np.ndarray:
    try:
        out, _ = _run(inputs, repeat=1, n_calls=1)
    except Exception:
        # transient device wedges have been observed; one retry
        out, _ = _run(inputs, repeat=1, n_calls=1)
    return out
